# revision 5
# baseline (speedup 1.0000x reference)
"""Trainium2 Bass kernel for the fused Lucy RNN cell.

Math (per batch row b, hidden dim d):
    z = x @ W.T + b                                  # [B,T,6D] projection
    s_t = sig(z0_t) * s_{t-1} + sig(z1_t)*tanh(z2_t)
    h_t = sig(z3_t) * h_{t-1} + sig(z4_t)*tanh(z5_t + s_t)
    out = h (all t), plus final s_T

Strategy:
  - Data parallel over batch: B=32 -> 4 rows per core across 8 cores.
  - On-chip layout: partition dim = hidden d (4 blocks of 128), free dim
    = time. Per (b, dblock) "subtile" the 6 gate projections are computed
    by PE (float32r matmuls, PSUM-accumulated over DIN), evacuated by
    ScalarE with fused sigmoid/tanh, and the two linear recurrences run
    as single VectorE tensor_tensor_scan instructions along time
    (state = a*state + u), chained across time-chunks via tiny carry
    columns.
"""

import numpy as np

# Problem constants (hardcoded per contest rules).
B, T, DIN, D = 32, 2048, 512, 512
NCORES = 8
BLOC = B // NCORES            # 4 batch rows per core
IBLK = DIN // 128             # 4 input-dim blocks
DBLK = D // 128               # 4 hidden-dim blocks
NGATE = 6
OBLK = NGATE * D // 128       # 24 output blocks of the fused projection
L = 512                       # time-chunk length (free dim per matmul)
NCHUNK = T // L               # 4 chunks

_CACHE = {}


def _build_module():
    from contextlib import ExitStack

    import concourse.mybir as mybir
    import concourse.tile as tile
    from concourse import bacc

    f32 = mybir.dt.float32
    f32r = mybir.dt.float32r
    AF = mybir.ActivationFunctionType
    OP = mybir.AluOpType

    nc = bacc.Bacc("TRN2", target_bir_lowering=False, debug=False)

    # Per-core DRAM I/O (shapes are per-core shards, pre-laid-out on host).
    xT = nc.dram_tensor("xT", [BLOC, IBLK, 128, T], f32r, kind="ExternalInput").ap()
    Wt = nc.dram_tensor("Wt", [IBLK, 128, NGATE * D], f32r, kind="ExternalInput").ap()
    bT = nc.dram_tensor("bT", [128, OBLK], f32, kind="ExternalInput").ap()
    h0T = nc.dram_tensor("h0T", [BLOC, DBLK, 128, 1], f32, kind="ExternalInput").ap()
    s0T = nc.dram_tensor("s0T", [BLOC, DBLK, 128, 1], f32, kind="ExternalInput").ap()
    hT = nc.dram_tensor("hT", [BLOC, DBLK, 128, T], f32, kind="ExternalOutput").ap()
    sT = nc.dram_tensor("sT", [BLOC, DBLK, 128, 1], f32, kind="ExternalOutput").ap()

    with tile.TileContext(nc) as tc, ExitStack() as ctx:
        wpool = ctx.enter_context(tc.tile_pool(name="w", bufs=1))
        xpool = ctx.enter_context(tc.tile_pool(name="x", bufs=2))
        gpool = ctx.enter_context(tc.tile_pool(name="g", bufs=2))
        cpool = ctx.enter_context(tc.tile_pool(name="carry", bufs=1))
        pspool = ctx.enter_context(tc.tile_pool(name="ps", bufs=8, space="PSUM"))

        # Weights resident in SBUF for the whole kernel: [128(i), iblk, 6D].
        # Fine-grained loads ordered (dk, g, i) so the first subtile's
        # weight set lands first and matmuls can start early.
        wsb = wpool.tile([128, IBLK, NGATE * D], f32r, name="wsb")
        bias = wpool.tile([128, OBLK], f32, name="bias")
        nc.sync.dma_start(bias[:], bT)
        for dk in range(DBLK):
            for g in range(NGATE):
                ob = g * DBLK + dk
                osl = slice(ob * 128, (ob + 1) * 128)
                for i in range(IBLK):
                    nc.sync.dma_start(wsb[:, i, osl], Wt[i, :, osl])

        # Carry state, one column per (b, dblk) subtile.
        s_carry = cpool.tile([128, BLOC * DBLK], f32, name="s_carry")
        h_carry = cpool.tile([128, BLOC * DBLK], f32, name="h_carry")
        for b in range(BLOC):
            for dk in range(DBLK):
                idx = b * DBLK + dk
                nc.sync.dma_start(s_carry[:, idx : idx + 1], s0T[b, dk])
                nc.sync.dma_start(h_carry[:, idx : idx + 1], h0T[b, dk])

        for ck in range(NCHUNK):
            tsl = slice(ck * L, (ck + 1) * L)
            # Load the x chunk for all (b, iblk): [128(i), iblk, b, L].
            xc = xpool.tile([128, IBLK, BLOC, L], f32r, name="xc")
            for b in range(BLOC):
                for i in range(IBLK):
                    nc.sync.dma_start(xc[:, i, b, :], xT[b, i, :, tsl])

            for dk in range(DBLK):
                for b in range(BLOC):
                    idx = b * DBLK + dk
                    # --- projection: 6 gates x 4 K-blocks of matmul ---
                    ps = []
                    for g in range(NGATE):
                        ob = g * DBLK + dk
                        pt = pspool.tile([128, L], f32, name=f"z{g}", tag="ps")
                        for i in range(IBLK):
                            nc.tensor.matmul(
                                pt[:],
                                wsb[:, i, ob * 128 : (ob + 1) * 128],
                                xc[:, i, b, :],
                                start=(i == 0),
                                stop=(i == IBLK - 1),
                            )
                        ps.append(pt)

                    def evac(g, func, name, nbufs=3):
                        t = gpool.tile([128, L], f32, name=name, bufs=nbufs)
                        ob = g * DBLK + dk
                        nc.scalar.activation(
                            t[:], ps[g][:], func, bias=bias[:, ob : ob + 1]
                        )
                        return t

                    # --- gates (ScalarE evacuates PSUM with fused LUT) ---
                    a = evac(0, AF.Sigmoid, "a")
                    g1 = evac(1, AF.Sigmoid, "g1")
                    g2 = evac(2, AF.Tanh, "g2")
                    c = evac(3, AF.Sigmoid, "c")
                    d = evac(4, AF.Sigmoid, "d")
                    # z5 has no LUT, but evacuate it immediately (Identity +
                    # bias) so its PSUM bank isn't held through the scan chain.
                    e0 = evac(5, AF.Identity, "e0")
                    u = gpool.tile([128, L], f32, name="u", bufs=3)
                    nc.gpsimd.tensor_tensor(u[:], g1[:], g2[:], op=OP.mult)

                    # --- s recurrence: s = a*s_prev + u along time ---
                    s = gpool.tile([128, L], f32, name="s", bufs=3)
                    nc.vector.tensor_tensor_scan(
                        s[:], a[:], u[:], s_carry[:, idx : idx + 1], OP.mult, OP.add
                    )
                    nc.vector.tensor_copy(s_carry[:, idx : idx + 1], s[:, L - 1 : L])

                    # --- h recurrence input: f = sig(z4)*tanh(z5 + s) ---
                    e = gpool.tile([128, L], f32, name="e", bufs=3)
                    nc.vector.tensor_tensor(e[:], e0[:], s[:], op=OP.add)
                    te = gpool.tile([128, L], f32, name="te", bufs=3)
                    nc.scalar.activation(te[:], e[:], AF.Tanh)
                    f = gpool.tile([128, L], f32, name="f", bufs=3)
                    nc.gpsimd.tensor_tensor(f[:], d[:], te[:], op=OP.mult)

                    # --- h recurrence: h = c*h_prev + f along time ---
                    h = gpool.tile([128, L], f32, name="h", bufs=3)
                    nc.vector.tensor_tensor_scan(
                        h[:], c[:], f[:], h_carry[:, idx : idx + 1], OP.mult, OP.add
                    )
                    nc.vector.tensor_copy(h_carry[:, idx : idx + 1], h[:, L - 1 : L])

                    nc.sync.dma_start(hT[b, dk, :, tsl], h[:])
                    if ck == NCHUNK - 1:
                        nc.sync.dma_start(sT[b, dk], s[:, L - 1 : L])

    nc.compile()
    return nc


def _get_module():
    if "nc" not in _CACHE:
        _CACHE["nc"] = _build_module()
    return _CACHE["nc"]


def kernel(x, h0, s0, W, b):
    from concourse.bass_utils import run_bass_kernel_spmd

    nc = _get_module()

    x = np.ascontiguousarray(x, dtype=np.float32)
    W = np.ascontiguousarray(W, dtype=np.float32)
    b = np.ascontiguousarray(b, dtype=np.float32)
    h0 = np.ascontiguousarray(h0, dtype=np.float32)
    s0 = np.ascontiguousarray(s0, dtype=np.float32)

    # Replicated weights, laid out for the PE: Wt[i, p, o] = W[o, i*128+p].
    Wt = np.ascontiguousarray(W.T).reshape(IBLK, 128, NGATE * D)
    bT = np.ascontiguousarray(b.reshape(OBLK, 128).T)

    in_maps = []
    for ci in range(NCORES):
        bs = slice(ci * BLOC, (ci + 1) * BLOC)
        xT = np.ascontiguousarray(x[bs].transpose(0, 2, 1)).reshape(
            BLOC, IBLK, 128, T
        )
        in_maps.append(
            {
                "xT": xT,
                "Wt": Wt,
                "bT": bT,
                "h0T": np.ascontiguousarray(h0[bs]).reshape(BLOC, DBLK, 128, 1),
                "s0T": np.ascontiguousarray(s0[bs]).reshape(BLOC, DBLK, 128, 1),
            }
        )

    res = run_bass_kernel_spmd(nc, in_maps, core_ids=list(range(NCORES))).results

    out = np.empty((B, T, D), np.float32)
    s_fin = np.empty((B, D), np.float32)
    for ci in range(NCORES):
        bs = slice(ci * BLOC, (ci + 1) * BLOC)
        out[bs] = res[ci]["hT"].transpose(0, 3, 1, 2).reshape(BLOC, T, D)
        s_fin[bs] = res[ci]["sT"].reshape(BLOC, D)
    return out, s_fin


if __name__ == "__main__":
    rng = np.random.default_rng(0)
    x = rng.normal(size=(B, T, DIN)).astype(np.float32)
    h0 = rng.normal(size=(B, D)).astype(np.float32)
    s0 = rng.normal(size=(B, D)).astype(np.float32)
    W = (rng.normal(size=(NGATE * D, DIN)) / np.sqrt(DIN)).astype(np.float32)
    b = np.zeros((NGATE * D,), np.float32)
    out, s_fin = kernel(x, h0, s0, W, b)
    print("out", out.shape, out.dtype, "sT", s_fin.shape)


# revision 7
# speedup vs baseline: 1.1466x; 1.1466x over previous
"""Trainium2 Bass kernel for the fused Lucy RNN cell.

Math (per batch row b, hidden dim d):
    z = x @ W.T + b                                  # [B,T,6D] projection
    s_t = sig(z0_t) * s_{t-1} + sig(z1_t)*tanh(z2_t)
    h_t = sig(z3_t) * h_{t-1} + sig(z4_t)*tanh(z5_t + s_t)
    out = h (all t), plus final s_T

Strategy:
  - Data parallel over batch: B=32 -> 4 rows per core across 8 cores.
  - On-chip layout: partition dim = hidden d (4 blocks of 128), free dim
    = time. Per (b, dblock) "subtile" the 6 gate projections are computed
    by PE (float32r matmuls at full rate, PSUM-accumulated over DIN),
    evacuated by ScalarE with fused sigmoid/tanh, and the two linear
    recurrences run as single VectorE tensor_tensor_scan instructions
    along time (state = a*state + u), chained across time-chunks via tiny
    carry columns.
  - The post-projection chain (u-mult, scans, tanh) is software-pipelined
    one subtile behind the matmul/evacuation phase so the ScalarE queue
    never head-of-line-blocks PSUM evacuation and PE streams without
    stalls.
"""

import numpy as np

# Problem constants (hardcoded per contest rules).
B, T, DIN, D = 32, 2048, 512, 512
NCORES = 8
BLOC = B // NCORES            # 4 batch rows per core
IBLK = DIN // 128             # 4 input-dim blocks
DBLK = D // 128               # 4 hidden-dim blocks
NGATE = 6
OBLK = NGATE * D // 128       # 24 output blocks of the fused projection
L = 512                       # time-chunk length (free dim per matmul)
NCHUNK = T // L               # 4 chunks

_CACHE = {}


def _build_module():
    from contextlib import ExitStack

    import concourse.mybir as mybir
    import concourse.tile as tile
    from concourse import bacc

    f32 = mybir.dt.float32
    f32r = mybir.dt.float32r
    AF = mybir.ActivationFunctionType
    OP = mybir.AluOpType

    nc = bacc.Bacc("TRN2", target_bir_lowering=False, debug=False)

    # Per-core DRAM I/O (shapes are per-core shards, pre-laid-out on host).
    xT = nc.dram_tensor("xT", [BLOC, IBLK, 128, T], f32r, kind="ExternalInput").ap()
    Wt = nc.dram_tensor("Wt", [IBLK, 128, NGATE * D], f32r, kind="ExternalInput").ap()
    bT = nc.dram_tensor("bT", [128, OBLK], f32, kind="ExternalInput").ap()
    h0P = nc.dram_tensor("h0P", [128, BLOC * DBLK], f32, kind="ExternalInput").ap()
    s0P = nc.dram_tensor("s0P", [128, BLOC * DBLK], f32, kind="ExternalInput").ap()
    hT = nc.dram_tensor("hT", [BLOC, DBLK, 128, T], f32, kind="ExternalOutput").ap()
    sP = nc.dram_tensor("sP", [128, BLOC * DBLK], f32, kind="ExternalOutput").ap()

    with tile.TileContext(nc) as tc, ExitStack() as ctx:
        wpool = ctx.enter_context(tc.tile_pool(name="w", bufs=1))
        xpool = ctx.enter_context(tc.tile_pool(name="x", bufs=2))
        gpool = ctx.enter_context(tc.tile_pool(name="g", bufs=3))
        cpool = ctx.enter_context(tc.tile_pool(name="carry", bufs=1))
        pspool = ctx.enter_context(tc.tile_pool(name="ps", bufs=8, space="PSUM"))

        # Weights resident in SBUF for the whole kernel: [128(i), iblk, 6D].
        wsb = wpool.tile([128, IBLK, NGATE * D], f32r, name="wsb")
        for i in range(IBLK):
            nc.sync.dma_start(wsb[:, i, :], Wt[i])

        # Small constants off the sync queue so they don't delay x loads.
        bias = wpool.tile([128, OBLK], f32, name="bias")
        nc.gpsimd.dma_start(bias[:], bT)
        s_carry = cpool.tile([128, BLOC * DBLK], f32, name="s_carry")
        h_carry = cpool.tile([128, BLOC * DBLK], f32, name="h_carry")
        nc.gpsimd.dma_start(s_carry[:], s0P)
        nc.gpsimd.dma_start(h_carry[:], h0P)

        xcs = {}

        def load_chunk(ck):
            tsl = slice(ck * L, (ck + 1) * L)
            xc = xpool.tile([128, IBLK, BLOC, L], f32r, name="xc", tag="xc")
            for b in range(BLOC):
                nc.sync.dma_start(
                    xc[:, :, b, :], xT[b].rearrange("i p t -> p i t")[:, :, tsl]
                )
            xcs[ck] = xc

        def phase1(j):
            """Matmuls + PSUM evacuation for subtile j."""
            ck, dk, b = j // 16, (j // 4) % 4, j % 4
            if j % 16 == 0:
                load_chunk(ck)
            xc = xcs[ck]
            ps = []
            for g in range(NGATE):
                ob = g * DBLK + dk
                pt = pspool.tile([128, L], f32, name=f"z{g}", tag="ps")
                for i in range(IBLK):
                    nc.tensor.matmul(
                        pt[:],
                        wsb[:, i, ob * 128 : (ob + 1) * 128],
                        xc[:, i, b, :],
                        start=(i == 0),
                        stop=(i == IBLK - 1),
                    )
                ps.append(pt)

            def evac(g, func, name):
                t = gpool.tile([128, L], f32, name=name)
                ob = g * DBLK + dk
                nc.scalar.activation(t[:], ps[g][:], func, bias=bias[:, ob : ob + 1])
                return t

            a = evac(0, AF.Sigmoid, "a")
            g1 = evac(1, AF.Sigmoid, "g1")
            g2 = evac(2, AF.Tanh, "g2")
            c = evac(3, AF.Sigmoid, "c")
            d = evac(4, AF.Sigmoid, "d")
            # z5 has no LUT, but evacuate it immediately (Identity + bias)
            # so its PSUM bank isn't held through the scan chain.
            e0 = evac(5, AF.Identity, "e0")
            u = gpool.tile([128, L], f32, name="u")
            nc.gpsimd.tensor_tensor(u[:], g1[:], g2[:], op=OP.mult)
            return dict(a=a, c=c, d=d, e0=e0, u=u, ck=ck, dk=dk, b=b)

        def phase2(st):
            """Scan chain + output DMA for a subtile (runs one step late)."""
            ck, dk, b = st["ck"], st["dk"], st["b"]
            idx = b * DBLK + dk
            tsl = slice(ck * L, (ck + 1) * L)

            # s recurrence: s = a*s_prev + u along time.
            s = gpool.tile([128, L], f32, name="s")
            nc.vector.tensor_tensor_scan(
                s[:], st["a"][:], st["u"][:], s_carry[:, idx : idx + 1],
                OP.mult, OP.add,
            )
            nc.vector.tensor_copy(s_carry[:, idx : idx + 1], s[:, L - 1 : L])

            # h recurrence input: f = sig(z4)*tanh(z5 + s).
            e = gpool.tile([128, L], f32, name="e")
            nc.vector.tensor_tensor(e[:], st["e0"][:], s[:], op=OP.add)
            te = gpool.tile([128, L], f32, name="te")
            nc.scalar.activation(te[:], e[:], AF.Tanh)
            f = gpool.tile([128, L], f32, name="f")
            nc.gpsimd.tensor_tensor(f[:], st["d"][:], te[:], op=OP.mult)

            # h recurrence: h = c*h_prev + f along time.
            h = gpool.tile([128, L], f32, name="h")
            nc.vector.tensor_tensor_scan(
                h[:], st["c"][:], f[:], h_carry[:, idx : idx + 1], OP.mult, OP.add
            )
            nc.vector.tensor_copy(h_carry[:, idx : idx + 1], h[:, L - 1 : L])

            nc.sync.dma_start(hT[b, dk, :, tsl], h[:])

        prev = None
        for j in range(NCHUNK * DBLK * BLOC):
            st = phase1(j)
            if prev is not None:
                phase2(prev)
            prev = st
        phase2(prev)

        # After the last chunk's carry copies, s_carry holds s_T.
        nc.sync.dma_start(sP[:], s_carry[:])

    nc.compile()
    return nc


def _get_module():
    if "nc" not in _CACHE:
        _CACHE["nc"] = _build_module()
    return _CACHE["nc"]


def make_in_maps(x, h0, s0, W, b):
    x = np.ascontiguousarray(x, dtype=np.float32)
    W = np.ascontiguousarray(W, dtype=np.float32)
    b = np.ascontiguousarray(b, dtype=np.float32)
    h0 = np.ascontiguousarray(h0, dtype=np.float32)
    s0 = np.ascontiguousarray(s0, dtype=np.float32)

    # Replicated weights, laid out for the PE: Wt[i, p, o] = W[o, i*128+p].
    Wt = np.ascontiguousarray(W.T).reshape(IBLK, 128, NGATE * D)
    bT = np.ascontiguousarray(b.reshape(OBLK, 128).T)

    in_maps = []
    for ci in range(NCORES):
        bs = slice(ci * BLOC, (ci + 1) * BLOC)
        xTl = np.ascontiguousarray(x[bs].transpose(0, 2, 1)).reshape(
            BLOC, IBLK, 128, T
        )
        # Carry layout: [p, b*DBLK + dk] = state[b, dk*128 + p].
        h0P = np.ascontiguousarray(h0[bs].reshape(BLOC, DBLK, 128).transpose(2, 0, 1))
        s0P = np.ascontiguousarray(s0[bs].reshape(BLOC, DBLK, 128).transpose(2, 0, 1))
        in_maps.append(
            {
                "xT": xTl,
                "Wt": Wt,
                "bT": bT,
                "h0P": h0P.reshape(128, BLOC * DBLK),
                "s0P": s0P.reshape(128, BLOC * DBLK),
            }
        )
    return in_maps


def kernel(x, h0, s0, W, b):
    from concourse.bass_utils import run_bass_kernel_spmd

    nc = _get_module()
    in_maps = make_in_maps(x, h0, s0, W, b)
    res = run_bass_kernel_spmd(nc, in_maps, core_ids=list(range(NCORES))).results

    out = np.empty((B, T, D), np.float32)
    s_fin = np.empty((B, D), np.float32)
    for ci in range(NCORES):
        bs = slice(ci * BLOC, (ci + 1) * BLOC)
        out[bs] = res[ci]["hT"].transpose(0, 3, 1, 2).reshape(BLOC, T, D)
        # sP[p, b*DBLK+dk] -> s[b, dk*128+p]
        s_fin[bs] = (
            res[ci]["sP"].reshape(128, BLOC, DBLK).transpose(1, 2, 0).reshape(BLOC, D)
        )
    return out, s_fin


if __name__ == "__main__":
    rng = np.random.default_rng(0)
    x = rng.normal(size=(B, T, DIN)).astype(np.float32)
    h0 = rng.normal(size=(B, D)).astype(np.float32)
    s0 = rng.normal(size=(B, D)).astype(np.float32)
    W = (rng.normal(size=(NGATE * D, DIN)) / np.sqrt(DIN)).astype(np.float32)
    b = np.zeros((NGATE * D,), np.float32)
    out, s_fin = kernel(x, h0, s0, W, b)
    print("out", out.shape, out.dtype, "sT", s_fin.shape)


# revision 8
# speedup vs baseline: 1.3591x; 1.1853x over previous
"""Trainium2 Bass kernel for the fused Lucy RNN cell.

Math (per batch row b, hidden dim d):
    z = x @ W.T + b                                  # [B,T,6D] projection
    s_t = sig(z0_t) * s_{t-1} + sig(z1_t)*tanh(z2_t)
    h_t = sig(z3_t) * h_{t-1} + sig(z4_t)*tanh(z5_t + s_t)
    out = h (all t), plus final s_T

Strategy:
  - Data parallel over batch: B=32 -> 4 rows per core across 8 cores.
  - On-chip layout: partition dim = hidden d (4 blocks of 128), free dim
    = time. Per (b, dblock) "subtile" the 6 gate projections are computed
    by PE (float32r matmuls at full rate, PSUM-accumulated over DIN),
    evacuated by ScalarE with fused sigmoid/tanh, and the two linear
    recurrences run as single VectorE tensor_tensor_scan instructions
    along time (state = a*state + u), chained across time-chunks via tiny
    carry columns.
  - The post-projection chain (u-mult, scans, tanh) is software-pipelined
    one subtile behind the matmul/evacuation phase so the ScalarE queue
    never head-of-line-blocks PSUM evacuation and PE streams without
    stalls.
"""

import numpy as np

# Problem constants (hardcoded per contest rules).
B, T, DIN, D = 32, 2048, 512, 512
NCORES = 8
BLOC = B // NCORES            # 4 batch rows per core
IBLK = DIN // 128             # 4 input-dim blocks
DBLK = D // 128               # 4 hidden-dim blocks
NGATE = 6
OBLK = NGATE * D // 128       # 24 output blocks of the fused projection
L = 512                       # time-chunk length (free dim per matmul)
NCHUNK = T // L               # 4 chunks

_CACHE = {}


def _build_module():
    from contextlib import ExitStack

    import concourse.mybir as mybir
    import concourse.tile as tile
    from concourse import bacc

    f32 = mybir.dt.float32
    f32r = mybir.dt.float32r
    AF = mybir.ActivationFunctionType
    OP = mybir.AluOpType

    nc = bacc.Bacc("TRN2", target_bir_lowering=False, debug=False)

    # Per-core DRAM I/O (shapes are per-core shards, pre-laid-out on host).
    xT = nc.dram_tensor("xT", [BLOC, IBLK, 128, T], f32r, kind="ExternalInput").ap()
    Wt = nc.dram_tensor("Wt", [IBLK, 128, NGATE * D], f32r, kind="ExternalInput").ap()
    bT = nc.dram_tensor("bT", [128, OBLK], f32, kind="ExternalInput").ap()
    h0P = nc.dram_tensor("h0P", [128, BLOC * DBLK], f32, kind="ExternalInput").ap()
    s0P = nc.dram_tensor("s0P", [128, BLOC * DBLK], f32, kind="ExternalInput").ap()
    hT = nc.dram_tensor("hT", [BLOC, DBLK, 128, T], f32, kind="ExternalOutput").ap()
    sP = nc.dram_tensor("sP", [128, BLOC * DBLK], f32, kind="ExternalOutput").ap()

    with tile.TileContext(nc) as tc, ExitStack() as ctx:
        wpool = ctx.enter_context(tc.tile_pool(name="w", bufs=1))
        xpool = ctx.enter_context(tc.tile_pool(name="x", bufs=2))
        gpool = ctx.enter_context(tc.tile_pool(name="g", bufs=4))
        cpool = ctx.enter_context(tc.tile_pool(name="carry", bufs=1))
        pspool = ctx.enter_context(tc.tile_pool(name="ps", bufs=8, space="PSUM"))

        # Weights resident in SBUF for the whole kernel: [128(i), iblk, 6D].
        wsb = wpool.tile([128, IBLK, NGATE * D], f32r, name="wsb")
        for i in range(IBLK):
            nc.sync.dma_start(wsb[:, i, :], Wt[i])

        # Small constants off the sync queue so they don't delay x loads.
        bias = wpool.tile([128, OBLK], f32, name="bias")
        nc.gpsimd.dma_start(bias[:], bT)
        s_carry = cpool.tile([128, BLOC * DBLK], f32, name="s_carry")
        h_carry = cpool.tile([128, BLOC * DBLK], f32, name="h_carry")
        nc.gpsimd.dma_start(s_carry[:], s0P)
        nc.gpsimd.dma_start(h_carry[:], h0P)

        xcs = {}

        def load_chunk(ck):
            tsl = slice(ck * L, (ck + 1) * L)
            for b in range(BLOC):
                xc = xpool.tile([128, IBLK, L], f32r, name="xc", tag="xc", bufs=6)
                nc.sync.dma_start(
                    xc[:], xT[b].rearrange("i p t -> p i t")[:, :, tsl]
                )
                xcs[(ck, b)] = xc

        def phase1(j):
            """Matmuls + PSUM evacuation for subtile j."""
            ck, dk, b = j // 16, (j // 4) % 4, j % 4
            if j % 16 == 0:
                load_chunk(ck)
            xc = xcs[(ck, b)]
            ps = []
            for g in range(NGATE):
                ob = g * DBLK + dk
                pt = pspool.tile([128, L], f32, name=f"z{g}", tag="ps")
                for i in range(IBLK):
                    nc.tensor.matmul(
                        pt[:],
                        wsb[:, i, ob * 128 : (ob + 1) * 128],
                        xc[:, i, :],
                        start=(i == 0),
                        stop=(i == IBLK - 1),
                    )
                ps.append(pt)

            def evac(g, func, name):
                t = gpool.tile([128, L], f32, name=name)
                ob = g * DBLK + dk
                nc.scalar.activation(t[:], ps[g][:], func, bias=bias[:, ob : ob + 1])
                return t

            a = evac(0, AF.Sigmoid, "a")
            g1 = evac(1, AF.Sigmoid, "g1")
            g2 = evac(2, AF.Tanh, "g2")
            c = evac(3, AF.Sigmoid, "c")
            d = evac(4, AF.Sigmoid, "d")
            # z5 has no LUT, but evacuate it immediately (Identity + bias)
            # so its PSUM bank isn't held through the scan chain.
            e0 = evac(5, AF.Identity, "e0")
            u = gpool.tile([128, L], f32, name="u")
            nc.vector.tensor_tensor(u[:], g1[:], g2[:], op=OP.mult)
            return dict(a=a, c=c, d=d, e0=e0, u=u, ck=ck, dk=dk, b=b)

        def phase2(st):
            """Scan chain + output DMA for a subtile (runs one step late)."""
            ck, dk, b = st["ck"], st["dk"], st["b"]
            idx = b * DBLK + dk
            tsl = slice(ck * L, (ck + 1) * L)

            # s recurrence: s = a*s_prev + u along time.
            s = gpool.tile([128, L], f32, name="s")
            nc.vector.tensor_tensor_scan(
                s[:], st["a"][:], st["u"][:], s_carry[:, idx : idx + 1],
                OP.mult, OP.add,
            )
            nc.vector.tensor_copy(s_carry[:, idx : idx + 1], s[:, L - 1 : L])

            # h recurrence input: f = sig(z4)*tanh(z5 + s).
            e = gpool.tile([128, L], f32, name="e")
            nc.vector.tensor_tensor(e[:], st["e0"][:], s[:], op=OP.add)
            te = gpool.tile([128, L], f32, name="te")
            nc.scalar.activation(te[:], e[:], AF.Tanh)
            f = gpool.tile([128, L], f32, name="f")
            nc.vector.tensor_tensor(f[:], st["d"][:], te[:], op=OP.mult)

            # h recurrence: h = c*h_prev + f along time.
            h = gpool.tile([128, L], f32, name="h")
            nc.vector.tensor_tensor_scan(
                h[:], st["c"][:], f[:], h_carry[:, idx : idx + 1], OP.mult, OP.add
            )
            nc.vector.tensor_copy(h_carry[:, idx : idx + 1], h[:, L - 1 : L])

            nc.sync.dma_start(hT[b, dk, :, tsl], h[:])

        prev = None
        for j in range(NCHUNK * DBLK * BLOC):
            st = phase1(j)
            if prev is not None:
                phase2(prev)
            prev = st
        phase2(prev)

        # After the last chunk's carry copies, s_carry holds s_T.
        nc.sync.dma_start(sP[:], s_carry[:])

    nc.compile()
    return nc


def _get_module():
    if "nc" not in _CACHE:
        _CACHE["nc"] = _build_module()
    return _CACHE["nc"]


def make_in_maps(x, h0, s0, W, b):
    x = np.ascontiguousarray(x, dtype=np.float32)
    W = np.ascontiguousarray(W, dtype=np.float32)
    b = np.ascontiguousarray(b, dtype=np.float32)
    h0 = np.ascontiguousarray(h0, dtype=np.float32)
    s0 = np.ascontiguousarray(s0, dtype=np.float32)

    # Replicated weights, laid out for the PE: Wt[i, p, o] = W[o, i*128+p].
    Wt = np.ascontiguousarray(W.T).reshape(IBLK, 128, NGATE * D)
    bT = np.ascontiguousarray(b.reshape(OBLK, 128).T)

    in_maps = []
    for ci in range(NCORES):
        bs = slice(ci * BLOC, (ci + 1) * BLOC)
        xTl = np.ascontiguousarray(x[bs].transpose(0, 2, 1)).reshape(
            BLOC, IBLK, 128, T
        )
        # Carry layout: [p, b*DBLK + dk] = state[b, dk*128 + p].
        h0P = np.ascontiguousarray(h0[bs].reshape(BLOC, DBLK, 128).transpose(2, 0, 1))
        s0P = np.ascontiguousarray(s0[bs].reshape(BLOC, DBLK, 128).transpose(2, 0, 1))
        in_maps.append(
            {
                "xT": xTl,
                "Wt": Wt,
                "bT": bT,
                "h0P": h0P.reshape(128, BLOC * DBLK),
                "s0P": s0P.reshape(128, BLOC * DBLK),
            }
        )
    return in_maps


def kernel(x, h0, s0, W, b):
    from concourse.bass_utils import run_bass_kernel_spmd

    nc = _get_module()
    in_maps = make_in_maps(x, h0, s0, W, b)
    res = run_bass_kernel_spmd(nc, in_maps, core_ids=list(range(NCORES))).results

    out = np.empty((B, T, D), np.float32)
    s_fin = np.empty((B, D), np.float32)
    for ci in range(NCORES):
        bs = slice(ci * BLOC, (ci + 1) * BLOC)
        out[bs] = res[ci]["hT"].transpose(0, 3, 1, 2).reshape(BLOC, T, D)
        # sP[p, b*DBLK+dk] -> s[b, dk*128+p]
        s_fin[bs] = (
            res[ci]["sP"].reshape(128, BLOC, DBLK).transpose(1, 2, 0).reshape(BLOC, D)
        )
    return out, s_fin


if __name__ == "__main__":
    rng = np.random.default_rng(0)
    x = rng.normal(size=(B, T, DIN)).astype(np.float32)
    h0 = rng.normal(size=(B, D)).astype(np.float32)
    s0 = rng.normal(size=(B, D)).astype(np.float32)
    W = (rng.normal(size=(NGATE * D, DIN)) / np.sqrt(DIN)).astype(np.float32)
    b = np.zeros((NGATE * D,), np.float32)
    out, s_fin = kernel(x, h0, s0, W, b)
    print("out", out.shape, out.dtype, "sT", s_fin.shape)


# revision 10
# speedup vs baseline: 1.4027x; 1.0320x over previous
"""Trainium2 Bass kernel for the fused Lucy RNN cell.

Math (per batch row b, hidden dim d):
    z = x @ W.T + b                                  # [B,T,6D] projection
    s_t = sig(z0_t) * s_{t-1} + sig(z1_t)*tanh(z2_t)
    h_t = sig(z3_t) * h_{t-1} + sig(z4_t)*tanh(z5_t + s_t)
    out = h (all t), plus final s_T

Strategy:
  - Data parallel over batch: B=32 -> 4 rows per core across 8 cores.
  - On-chip layout: partition dim = hidden d (4 blocks of 128), free dim
    = time. Per (b, dblock) "subtile" the 6 gate projections are computed
    by PE (float32r matmuls at full rate, PSUM-accumulated over DIN),
    evacuated by ScalarE with fused sigmoid/tanh, and the two linear
    recurrences run as single VectorE tensor_tensor_scan instructions
    along time (state = a*state + u), chained across time-chunks via tiny
    carry columns.
  - The post-projection chain (u-mult, scans, tanh) is software-pipelined
    one subtile behind the matmul/evacuation phase so the ScalarE queue
    never head-of-line-blocks PSUM evacuation and PE streams without
    stalls.
"""

import numpy as np

# Problem constants (hardcoded per contest rules).
B, T, DIN, D = 32, 2048, 512, 512
NCORES = 8
BLOC = B // NCORES            # 4 batch rows per core
IBLK = DIN // 128             # 4 input-dim blocks
DBLK = D // 128               # 4 hidden-dim blocks
NGATE = 6
OBLK = NGATE * D // 128       # 24 output blocks of the fused projection
L = 512                       # time-chunk length (free dim per matmul)
NCHUNK = T // L               # 4 chunks

_CACHE = {}


def _build_module():
    from contextlib import ExitStack

    import concourse.mybir as mybir
    import concourse.tile as tile
    from concourse import bacc

    f32 = mybir.dt.float32
    f32r = mybir.dt.float32r
    AF = mybir.ActivationFunctionType
    OP = mybir.AluOpType

    nc = bacc.Bacc("TRN2", target_bir_lowering=False, debug=False)

    # Per-core DRAM I/O (shapes are per-core shards, pre-laid-out on host).
    xT = nc.dram_tensor("xT", [BLOC, IBLK, 128, T], f32r, kind="ExternalInput").ap()
    Wt = nc.dram_tensor("Wt", [IBLK, 128, NGATE * D], f32r, kind="ExternalInput").ap()
    bT = nc.dram_tensor("bT", [128, OBLK], f32, kind="ExternalInput").ap()
    h0P = nc.dram_tensor("h0P", [128, BLOC * DBLK], f32, kind="ExternalInput").ap()
    s0P = nc.dram_tensor("s0P", [128, BLOC * DBLK], f32, kind="ExternalInput").ap()
    hT = nc.dram_tensor("hT", [BLOC, DBLK, 128, T], f32, kind="ExternalOutput").ap()
    sP = nc.dram_tensor("sP", [128, BLOC * DBLK], f32, kind="ExternalOutput").ap()

    with tile.TileContext(nc) as tc, ExitStack() as ctx:
        wpool = ctx.enter_context(tc.tile_pool(name="w", bufs=1))
        xpool = ctx.enter_context(tc.tile_pool(name="x", bufs=2))
        gpool = ctx.enter_context(tc.tile_pool(name="g", bufs=4))
        cpool = ctx.enter_context(tc.tile_pool(name="carry", bufs=1))
        pspool = ctx.enter_context(tc.tile_pool(name="ps", bufs=8, space="PSUM"))

        # Weights resident in SBUF for the whole kernel: [128(i), iblk, 6D].
        # Loaded in (dk, i) granules on separate DMA queues; dk=0 first so
        # the first subtile's matmuls can start early.
        wsb = wpool.tile([128, IBLK, NGATE * D], f32r, name="wsb")

        def load_w(dk):
            for i in range(IBLK):
                nc.sync.dma_start(
                    wsb[:, i, :].rearrange("p (g dk c) -> p g dk c", g=NGATE, dk=DBLK)[
                        :, :, dk, :
                    ],
                    Wt[i].rearrange("p (g dk c) -> p g dk c", g=NGATE, dk=DBLK)[
                        :, :, dk, :
                    ],
                )

        load_w(0)

        # Small constants off the sync queue so they don't delay x loads.
        bias = wpool.tile([128, OBLK], f32, name="bias")
        nc.gpsimd.dma_start(bias[:], bT)
        s_carry = cpool.tile([128, BLOC * DBLK], f32, name="s_carry")
        h_carry = cpool.tile([128, BLOC * DBLK], f32, name="h_carry")
        nc.gpsimd.dma_start(s_carry[:], s0P)
        nc.gpsimd.dma_start(h_carry[:], h0P)

        xcs = {}

        def load_chunk(ck):
            tsl = slice(ck * L, (ck + 1) * L)
            for b in range(BLOC):
                xc = xpool.tile([128, IBLK, L], f32r, name="xc", tag="xc", bufs=6)
                for i in range(IBLK):
                    nc.sync.dma_start(xc[:, i, :], xT[b, i, :, tsl])
                xcs[(ck, b)] = xc

        def phase1(j):
            """Matmuls + PSUM evacuation for subtile j."""
            ck, dk, b = j // 16, (j // 4) % 4, j % 4
            if j % 16 == 0:
                load_chunk(ck)
            if j == 1:
                for late_dk in range(1, DBLK):
                    load_w(late_dk)
            xc = xcs[(ck, b)]
            ps = []
            for g in range(NGATE):
                ob = g * DBLK + dk
                pt = pspool.tile([128, L], f32, name=f"z{g}", tag="ps")
                for i in range(IBLK):
                    nc.tensor.matmul(
                        pt[:],
                        wsb[:, i, ob * 128 : (ob + 1) * 128],
                        xc[:, i, :],
                        start=(i == 0),
                        stop=(i == IBLK - 1),
                    )
                ps.append(pt)

            def evac(g, func, name):
                t = gpool.tile([128, L], f32, name=name)
                ob = g * DBLK + dk
                nc.scalar.activation(t[:], ps[g][:], func, bias=bias[:, ob : ob + 1])
                return t

            a = evac(0, AF.Sigmoid, "a")
            g1 = evac(1, AF.Sigmoid, "g1")
            g2 = evac(2, AF.Tanh, "g2")
            c = evac(3, AF.Sigmoid, "c")
            d = evac(4, AF.Sigmoid, "d")
            # z5 has no LUT, but evacuate it immediately (Identity + bias)
            # so its PSUM bank isn't held through the scan chain.
            e0 = evac(5, AF.Identity, "e0")
            u = gpool.tile([128, L], f32, name="u")
            nc.vector.tensor_tensor(u[:], g1[:], g2[:], op=OP.mult)
            return dict(a=a, c=c, d=d, e0=e0, u=u, ck=ck, dk=dk, b=b)

        def phase2(st):
            """Scan chain + output DMA for a subtile (runs one step late)."""
            ck, dk, b = st["ck"], st["dk"], st["b"]
            idx = b * DBLK + dk
            tsl = slice(ck * L, (ck + 1) * L)

            # s recurrence: s = a*s_prev + u along time.
            s = gpool.tile([128, L], f32, name="s")
            nc.vector.tensor_tensor_scan(
                s[:], st["a"][:], st["u"][:], s_carry[:, idx : idx + 1],
                OP.mult, OP.add,
            )
            nc.vector.tensor_copy(s_carry[:, idx : idx + 1], s[:, L - 1 : L])

            # h recurrence input: f = sig(z4)*tanh(z5 + s).
            e = gpool.tile([128, L], f32, name="e")
            nc.vector.tensor_tensor(e[:], st["e0"][:], s[:], op=OP.add)
            te = gpool.tile([128, L], f32, name="te")
            nc.scalar.activation(te[:], e[:], AF.Tanh)
            f = gpool.tile([128, L], f32, name="f")
            nc.vector.tensor_tensor(f[:], st["d"][:], te[:], op=OP.mult)

            # h recurrence: h = c*h_prev + f along time.
            h = gpool.tile([128, L], f32, name="h")
            nc.vector.tensor_tensor_scan(
                h[:], st["c"][:], f[:], h_carry[:, idx : idx + 1], OP.mult, OP.add
            )
            nc.vector.tensor_copy(h_carry[:, idx : idx + 1], h[:, L - 1 : L])

            if ck == NCHUNK - 1 and b == BLOC - 1 and dk == DBLK - 1:
                for q in range(4):
                    qsl = slice(q * (L // 4), (q + 1) * (L // 4))
                    nc.sync.dma_start(
                        hT[b, dk, :, ck * L + q * (L // 4) : ck * L + (q + 1) * (L // 4)],
                        h[:, qsl],
                    )
            else:
                nc.sync.dma_start(hT[b, dk, :, tsl], h[:])

        # Warm the PE HAM clock gate during the initial DMA wait so the
        # first real matmuls run at 2.4 GHz. Uses the first W granule as
        # both operands (results are discarded).
        pwu = pspool.tile([128, 128], f32, name="pwu", tag="ps")
        for _ in range(24):
            nc.tensor.matmul(
                pwu[:], wsb[:, 0, 0:128], wsb[:, 0, 128:256], start=True, stop=True
            )

        prev = None
        for j in range(NCHUNK * DBLK * BLOC):
            st = phase1(j)
            if prev is not None:
                phase2(prev)
            prev = st
        phase2(prev)

        # After the last chunk's carry copies, s_carry holds s_T.
        nc.sync.dma_start(sP[:], s_carry[:])

    nc.compile()
    return nc


def _get_module():
    if "nc" not in _CACHE:
        _CACHE["nc"] = _build_module()
    return _CACHE["nc"]


def make_in_maps(x, h0, s0, W, b):
    x = np.ascontiguousarray(x, dtype=np.float32)
    W = np.ascontiguousarray(W, dtype=np.float32)
    b = np.ascontiguousarray(b, dtype=np.float32)
    h0 = np.ascontiguousarray(h0, dtype=np.float32)
    s0 = np.ascontiguousarray(s0, dtype=np.float32)

    # Replicated weights, laid out for the PE: Wt[i, p, o] = W[o, i*128+p].
    Wt = np.ascontiguousarray(W.T).reshape(IBLK, 128, NGATE * D)
    bT = np.ascontiguousarray(b.reshape(OBLK, 128).T)

    in_maps = []
    for ci in range(NCORES):
        bs = slice(ci * BLOC, (ci + 1) * BLOC)
        xTl = np.ascontiguousarray(x[bs].transpose(0, 2, 1)).reshape(
            BLOC, IBLK, 128, T
        )
        # Carry layout: [p, b*DBLK + dk] = state[b, dk*128 + p].
        h0P = np.ascontiguousarray(h0[bs].reshape(BLOC, DBLK, 128).transpose(2, 0, 1))
        s0P = np.ascontiguousarray(s0[bs].reshape(BLOC, DBLK, 128).transpose(2, 0, 1))
        in_maps.append(
            {
                "xT": xTl,
                "Wt": Wt,
                "bT": bT,
                "h0P": h0P.reshape(128, BLOC * DBLK),
                "s0P": s0P.reshape(128, BLOC * DBLK),
            }
        )
    return in_maps


def kernel(x, h0, s0, W, b):
    from concourse.bass_utils import run_bass_kernel_spmd

    nc = _get_module()
    in_maps = make_in_maps(x, h0, s0, W, b)
    res = run_bass_kernel_spmd(nc, in_maps, core_ids=list(range(NCORES))).results

    out = np.empty((B, T, D), np.float32)
    s_fin = np.empty((B, D), np.float32)
    for ci in range(NCORES):
        bs = slice(ci * BLOC, (ci + 1) * BLOC)
        out[bs] = res[ci]["hT"].transpose(0, 3, 1, 2).reshape(BLOC, T, D)
        # sP[p, b*DBLK+dk] -> s[b, dk*128+p]
        s_fin[bs] = (
            res[ci]["sP"].reshape(128, BLOC, DBLK).transpose(1, 2, 0).reshape(BLOC, D)
        )
    return out, s_fin


if __name__ == "__main__":
    rng = np.random.default_rng(0)
    x = rng.normal(size=(B, T, DIN)).astype(np.float32)
    h0 = rng.normal(size=(B, D)).astype(np.float32)
    s0 = rng.normal(size=(B, D)).astype(np.float32)
    W = (rng.normal(size=(NGATE * D, DIN)) / np.sqrt(DIN)).astype(np.float32)
    b = np.zeros((NGATE * D,), np.float32)
    out, s_fin = kernel(x, h0, s0, W, b)
    print("out", out.shape, out.dtype, "sT", s_fin.shape)
